# revision 5
# baseline (speedup 1.0000x reference)
"""Causal multi-head attention on 8 Trainium2 NeuronCores.

Problem: X[4,2048,1024] @ {W_q,W_k,W_v}[1024,1024], 16 heads, causal softmax
(scale = sqrt(1024)), output [4,2048,1024] fp32.

Sharding (8 cores): core c = (batch b = c//2, head-group g = c%2).
Each core handles one batch and 8 heads (W columns 512*g : 512*(g+1)),
producing output columns 512*g : 512*(g+1) of its batch. Outputs are fully
disjoint -> no collectives; inputs are sliced on host.

Per-core kernel (Tile framework), fully on-chip (no DRAM round trip for P):
  Phase A: Q^T, K^T = W^T @ X^T (f32r matmuls), V = X @ W_v natural layout,
           cast fp16.
  Phase B per head, per 128-query block:
    - scores panel in PSUM (f32r, K=64), causal mask add on the diagonal
      block, DVE row-max;
    - ACT exp(scale*s - scale*max) -> P fp16 in SBUF, row-sums free via
      accum_out; DVE row-normalize P in place (4x fp16 mode);
    - PE-transpose P 128x128 blocks -> PSUM fp16, copy into persistent
      P^T tile (partitions = k, free = q);
    - AV: out^T[dv,q] = sum_kb V[kb]^T-stationary matmuls over P^T in PSUM;
      PE-transpose back to [q,dv] and DMA out. No per-element rescale needed
      at the end because P was pre-normalized.
"""

import numpy as np

D = 1024          # model dim
S = 2048          # sequence length
HD = 512          # head-columns per core (8 heads x 64)
NH = 8            # heads per core
DH = 64           # head dim
SCALE = 1.0 / 32.0  # 1/sqrt(QK_DIM)
NEG_BIG = -1.0e30

_CACHE = {}
LAST_RESULTS = None


def _build_nc(repeat=1):
    import concourse.bacc as bacc
    import concourse.mybir as mybir
    from concourse.tile import TileContext
    from concourse.tile_rust import add_dep_helper
    from concourse.masks import make_identity, make_causal_mask

    F32 = mybir.dt.float32
    F32R = mybir.dt.float32r
    F16 = mybir.dt.float16
    Exp = mybir.ActivationFunctionType.Exp
    AX = mybir.AxisListType.X
    MAX = mybir.AluOpType.max
    ADD = mybir.AluOpType.add

    nc = bacc.Bacc("TRN2", target_bir_lowering=False, debug=False, num_devices=8)
    XT = nc.dram_tensor("XT", [D, S], F32R, kind="ExternalInput").ap()
    WQ = nc.dram_tensor("WQ", [D, HD], F32R, kind="ExternalInput").ap()
    WK = nc.dram_tensor("WK", [D, HD], F32R, kind="ExternalInput").ap()
    WV = nc.dram_tensor("WV", [D, HD], F32R, kind="ExternalInput").ap()
    O = nc.dram_tensor("O", [S, HD], F32, kind="ExternalOutput").ap()

    with TileContext(nc) as tc:
        with tc.tile_pool(name="persist", bufs=1) as pp, \
             tc.tile_pool(name="small", bufs=2) as smp:
            qt = [pp.tile([128, S], F32R, tag=f"qt{m}", name=f"qt{m}") for m in range(4)]
            kt = [pp.tile([128, S], F32R, tag=f"kt{m}", name=f"kt{m}") for m in range(4)]
            vt = [pp.tile([128, HD], F16, tag=f"v{st}", name=f"v{st}") for st in range(16)]
            identp = pp.tile([128, 128], F16, tag="identp", name="identp")
            make_identity(nc, identp)
            idento = pp.tile([64, 64], F32, tag="idento", name="idento")
            make_identity(nc, idento)
            cmask = pp.tile([128, 128], F32, tag="cmask", name="cmask")
            make_causal_mask(nc, cmask, mask_val=NEG_BIG)

            for _rep in range(repeat):
                # ---------------- Phase A: projections ----------------
                with tc.tile_pool(name="phaseA", bufs=1) as pa, \
                     tc.tile_pool(name="projps", bufs=1, space="PSUM") as pps:
                    w_sb = {}
                    for wname, W in (("q", WQ), ("k", WK), ("v", WV)):
                        for kc in range(8):
                            t = pa.tile([128, HD], F32R, tag=f"w{wname}{kc}",
                                        name=f"w{wname}{kc}")
                            nc.sync.dma_start(out=t, in_=W[128 * kc:128 * (kc + 1), :])
                            w_sb[(wname, kc)] = t
                    phase_a_tail = []
                    for half in range(2):
                        off = 1024 * half
                        xt_sb = []
                        for kc in range(8):
                            t = pa.tile([128, 1024], F32R, tag=f"xt{kc}", bufs=2,
                                        name=f"xt{kc}_{half}")
                            nc.sync.dma_start(
                                out=t, in_=XT[128 * kc:128 * (kc + 1), off:off + 1024])
                            xt_sb.append(t)
                        # Q^T and K^T: [head-cols 128m.., seq] chunks
                        for wname, dest in (("q", qt), ("k", kt)):
                            for m in range(4):
                                psl = [pps.tile([128, 512], F32, tag="projqk", bufs=5,
                                                name=f"ps{wname}{half}{m}{n}")
                                       for n in range(2)]
                                for kc in range(8):
                                    for n in range(2):
                                        nc.tensor.matmul(
                                            psl[n],
                                            lhsT=w_sb[(wname, kc)][:, 128 * m:128 * (m + 1)],
                                            rhs=xt_sb[kc][:, 512 * n:512 * (n + 1)],
                                            start=(kc == 0), stop=(kc == 7))
                                for n in range(2):
                                    cp = nc.vector.tensor_copy(
                                        dest[m][:, off + 512 * n:off + 512 * (n + 1)],
                                        psl[n])
                                    if half == 1:
                                        phase_a_tail.append(cp)
                        # V natural layout [seq, head-cols], fp16
                        for stl in range(8):
                            st = 8 * half + stl
                            psv = pps.tile([128, 512], F32, tag="projv", bufs=2,
                                           name=f"psv{st}")
                            for kc in range(8):
                                nc.tensor.matmul(
                                    psv,
                                    lhsT=xt_sb[kc][:, 128 * stl:128 * (stl + 1)],
                                    rhs=w_sb[("v", kc)],
                                    start=(kc == 0), stop=(kc == 7))
                            cpv = nc.scalar.copy(vt[st], psv)
                            if half == 1:
                                phase_a_tail.append(cpv)

                # ---------------- Phase B: attention ----------------
                with tc.tile_pool(name="phaseB", bufs=1) as pb, \
                     tc.tile_pool(name="bps", bufs=1, space="PSUM") as bps:
                    # P^T for the current head: partitions = k within block kb,
                    # free = (kb, q). Zero regions (q < 128*kb) memset once:
                    # no head ever writes them, so full-range AV reads stay exact.
                    PT = pb.tile([128, 16, S], F16, tag="PT", name="PT")
                    for kb in range(1, 16):
                        nc.gpsimd.memset(PT[:, kb, 0:128 * kb], 0.0)
                    for h in range(NH):
                        mt = h // 2
                        po = 64 * (h % 2)
                        sums = pb.tile([128, 16], F32, tag="sums", bufs=2,
                                       name=f"sums{h}")
                        for qb in range(16):
                            nk = 128 * (qb + 1)
                            nsp = (nk + 1023) // 1024
                            pans = []
                            mxs = []
                            for spi in range(nsp):
                                klo = 1024 * spi
                                kw = min(1024, nk - klo)
                                pan = bps.tile([128, 1024], F32, tag="pan", bufs=2,
                                               name=f"pan{h}_{qb}_{spi}")
                                for c in range((kw + 511) // 512):
                                    cw = min(512, kw - 512 * c)
                                    mm = nc.tensor.matmul(
                                        pan[:, 512 * c:512 * c + cw],
                                        lhsT=qt[mt][po:po + 64, 128 * qb:128 * (qb + 1)],
                                        rhs=kt[mt][po:po + 64,
                                                   klo + 512 * c:klo + 512 * c + cw],
                                        start=True, stop=True)
                                    if h == 0 and qb == 0:
                                        # phase boundary: PSUM slots are reused
                                        # across pools; PE stream is in-order, so
                                        # gating the first phase-B matmul on all
                                        # phase-A PSUM readers orders everything.
                                        for cp in phase_a_tail:
                                            add_dep_helper(
                                                mm.ins, cp.ins,
                                                reason="phaseA-psum drain")
                                mx = smp.tile([128, 1], F32, tag="mx", bufs=8,
                                              name=f"mx{h}_{qb}_{spi}")
                                if klo + kw == nk:
                                    nc.vector.tensor_tensor(
                                        out=pan[:, kw - 128:kw],
                                        in0=pan[:, kw - 128:kw], in1=cmask, op=ADD)
                                nc.vector.tensor_reduce(mx, pan[:, 0:kw],
                                                        axis=AX, op=MAX)
                                pans.append((pan, klo, kw))
                                mxs.append(mx)
                            if nsp == 2:
                                mxc = smp.tile([128, 1], F32, tag="mxc", bufs=4,
                                               name=f"mxc{h}_{qb}")
                                nc.vector.tensor_tensor(out=mxc, in0=mxs[0],
                                                        in1=mxs[1], op=MAX)
                            else:
                                mxc = mxs[0]
                            bias = smp.tile([128, 1], F32, tag="bias", bufs=4,
                                            name=f"bias{h}_{qb}")
                            nc.vector.tensor_scalar_mul(bias, mxc, -SCALE)
                            P = pb.tile([128, S], F16, tag="pexp", bufs=3,
                                        name=f"p{h}_{qb}")
                            accs = []
                            for spi, (pan, klo, kw) in enumerate(pans):
                                if nsp == 1:
                                    acc = sums[:, qb:qb + 1]
                                else:
                                    acc = smp.tile([128, 1], F32, tag="acc", bufs=4,
                                                   name=f"acc{h}_{qb}_{spi}")
                                    accs.append(acc)
                                nc.scalar.activation(P[:, klo:klo + kw], pan[:, 0:kw],
                                                     Exp, bias=bias, scale=SCALE,
                                                     accum_out=acc)
                            if nsp == 2:
                                nc.vector.tensor_tensor(out=sums[:, qb:qb + 1],
                                                        in0=accs[0], in1=accs[1],
                                                        op=ADD)
                            # normalize P rows in place (fp16 SBUF single-src: 4x)
                            rc = smp.tile([128, 1], F32, tag="rc", bufs=4,
                                          name=f"rc{h}_{qb}")
                            nc.vector.reciprocal(rc, sums[:, qb:qb + 1])
                            nc.vector.tensor_scalar_mul(P[:, 0:nk], P[:, 0:nk], rc)
                            # PE-transpose P 128x128 blocks into PT
                            for g in range((qb + 8) // 8):
                                n = min(8, (qb + 1) - 8 * g)
                                ptp = bps.tile([128, 1024], F16, tag="ptp", bufs=2,
                                               name=f"ptp{h}_{qb}_{g}")
                                for w in range(n):
                                    j = 8 * g + w
                                    nc.tensor.transpose(
                                        ptp[:, 128 * w:128 * (w + 1)],
                                        P[:, 128 * j:128 * (j + 1)], identp)
                                src = ptp[:, 0:128 * n].rearrange(
                                    "p (j f) -> p j f", f=128)
                                dst = PT[:, 8 * g:8 * g + n, 128 * qb:128 * (qb + 1)]
                                if (qb + g) % 2 == 0:
                                    nc.scalar.copy(dst, src)
                                else:
                                    nc.vector.tensor_copy(dst, src)
                        # AV: out^T[dv, q] accumulated over k-blocks in PSUM
                        for qc in range(4):
                            avp = bps.tile([64, 512], F32, tag="av", bufs=1,
                                           name=f"avp{h}_{qc}")
                            nkb = 4 * qc + 4
                            for kb in range(nkb):
                                nc.tensor.matmul(
                                    avp,
                                    lhsT=vt[kb][:, 64 * h:64 * (h + 1)],
                                    rhs=PT[:, kb, 512 * qc:512 * (qc + 1)],
                                    start=(kb == 0), stop=(kb == nkb - 1))
                            outT = pb.tile([64, 512], F32, tag="outT", bufs=2,
                                           name=f"outT{h}_{qc}")
                            nc.scalar.copy(outT, avp)
                            tps = bps.tile([128, 256], F32, tag="outtr", bufs=1,
                                           name=f"tps{h}_{qc}")
                            for j in range(4):
                                nc.tensor.transpose(
                                    tps[:, 64 * j:64 * (j + 1)],
                                    outT[:, 128 * j:128 * (j + 1)], idento)
                            ob = pb.tile([128, 4, 64], F32, tag="ob", bufs=2,
                                         name=f"ob{h}_{qc}")
                            nc.vector.tensor_copy(
                                ob, tps.rearrange("p (j f) -> p j f", f=64))
                            dst = O[512 * qc:512 * (qc + 1),
                                    64 * h:64 * (h + 1)].rearrange("(j p) f -> p j f",
                                                                   p=128)
                            nc.sync.dma_start(out=dst, in_=ob)

    nc.compile()
    return nc


def _get_runner(repeat=1):
    """Build nc once, wrap it in a jitted shard_map over 8 cores.

    Mirrors concourse.bass2jax.run_bass_via_pjrt but without output-buffer
    donation so the compiled callable can be re-invoked on device-resident
    inputs for timing.
    """
    key = ("runner", repeat)
    if key in _CACHE:
        return _CACHE[key]

    import jax
    from jax.sharding import Mesh, PartitionSpec
    from jax.experimental.shard_map import shard_map
    import concourse.mybir as mybir
    from concourse import bass2jax

    nc = _build_nc(repeat)
    _CACHE["nc_obj"] = nc
    bass2jax.install_neuronx_cc_hook()

    partition_name = (nc.partition_id_tensor.name
                      if nc.partition_id_tensor else None)
    in_names = []
    out_names = []
    out_avals = []
    for alloc in nc.m.functions[0].allocations:
        if not isinstance(alloc, mybir.MemoryLocationSet):
            continue
        name = alloc.memorylocations[0].name
        if alloc.kind == "ExternalInput":
            if name != partition_name:
                in_names.append(name)
        elif alloc.kind == "ExternalOutput":
            out_names.append(name)
            out_avals.append(jax.core.ShapedArray(
                tuple(alloc.tensor_shape), mybir.dt.np(alloc.dtype)))
    n_params = len(in_names)
    all_names = in_names + out_names
    if partition_name is not None:
        all_names = all_names + [partition_name]

    def _body(*args):
        operands = list(args)
        if partition_name is not None:
            operands.append(bass2jax.partition_id_tensor())
        outs = bass2jax._bass_exec_p.bind(
            *operands,
            out_avals=tuple(out_avals),
            in_names=tuple(all_names),
            out_names=tuple(out_names),
            lowering_input_output_aliases=(),
            sim_require_finite=True,
            sim_require_nnan=True,
            nc=nc,
        )
        return tuple(outs)

    devices = jax.devices()[:8]
    mesh = Mesh(np.asarray(devices), ("core",))
    n_out = len(out_names)
    sharded = jax.jit(
        shard_map(_body, mesh=mesh,
                  in_specs=(PartitionSpec("core"),) * (n_params + n_out),
                  out_specs=(PartitionSpec("core"),) * n_out,
                  check_rep=False),
        keep_unused=True,
    )
    _CACHE[key] = (sharded, in_names, out_names, out_avals)
    return _CACHE[key]


def _prepare_dev_args(X, W_q, W_k, W_v, repeat=1):
    import jax

    sharded, in_names, out_names, out_avals = _get_runner(repeat)
    per_core = {name: [] for name in in_names}
    for c in range(8):
        b, g = c // 2, c % 2
        cols = slice(HD * g, HD * (g + 1))
        vals = {
            "XT": np.ascontiguousarray(X[b].T),
            "WQ": np.ascontiguousarray(W_q[:, cols]),
            "WK": np.ascontiguousarray(W_k[:, cols]),
            "WV": np.ascontiguousarray(W_v[:, cols]),
        }
        for name in in_names:
            per_core[name].append(vals[name])
    args = [np.concatenate(per_core[name], axis=0) for name in in_names]
    for aval in out_avals:
        args.append(np.zeros((8 * aval.shape[0], *aval.shape[1:]), aval.dtype))
    return args


def kernel(X, W_q, W_k, W_v):
    global LAST_RESULTS
    X = np.asarray(X, dtype=np.float32)
    W_q = np.asarray(W_q, dtype=np.float32)
    W_k = np.asarray(W_k, dtype=np.float32)
    W_v = np.asarray(W_v, dtype=np.float32)
    B = X.shape[0]

    sharded, in_names, out_names, out_avals = _get_runner()
    args = _prepare_dev_args(X, W_q, W_k, W_v)
    out_arrs = sharded(*args)
    LAST_RESULTS = (sharded, args)

    o_idx = out_names.index("O")
    o_full = np.asarray(out_arrs[o_idx]).reshape(8, S, HD)
    out = np.empty((B, S, D), dtype=np.float32)
    for c in range(8):
        b, g = c // 2, c % 2
        out[b, :, HD * g:HD * (g + 1)] = o_full[c]
    return out
